# revision 1
# baseline (speedup 1.0000x reference)
"""Distributed cross-entropy loss kernel for Trainium2 (8 NeuronCores).

loss = -mean_t(log_softmax(h @ E^T + b)[t, labels[t]])
     = mean_t(LSE_t) - mean_t(h_t . E[labels[t]] + b[labels[t]])

Strategy (variant c): vocab tensor parallel across 8 cores; the host
pre-transposes and pre-quantizes both matmul operands to fp8e4 so the
device does nothing but fp8 DoubleRow matmuls, per-partition-biased
exp on the scalar engine, and fp8 DoubleRow ones-matmul partition
reductions. A tiny AllReduce combines per-core sumexp/target partials
and every core finishes the log + mean locally.

Orientation: vocab on PSUM partitions, tokens moving. Per (vocab-tile,
token-group) the PE accumulates K=1024 in 4 DoubleRow passes, ScalarE
applies exp(logit + b_v - ln8) writing fp8, and pairs of vocab tiles
are reduced over partitions with a DoubleRow ones-matmul into 8
persistent [1,512] PSUM accumulators (3 banks, offsets 0/32/64).

exp is pre-scaled by 1/8 (bias shift of -ln 8) so values fit e4m3's
max of 448; the final loss adds ln 8 back.
"""

from contextlib import ExitStack

import numpy as np
import ml_dtypes

import concourse.bass as bass
import concourse.tile as tile
from concourse import bacc, mybir

F32 = mybir.dt.float32
BF16 = mybir.dt.bfloat16
FP8 = mybir.dt.float8e4
I32 = mybir.dt.int32
AF = mybir.ActivationFunctionType
ALU = mybir.AluOpType
DR = mybir.MatmulPerfMode.DoubleRow

P = 128

# fp8 operand scaling: h' = ALPHA*h, E' = BETA*E with ALPHA*BETA == 1, so
# logits keep their true scale. Balancing puts both operands at ~0.18 std,
# inside e4m3's normal range (h ~ N(0,1), E rows ~ N(0, 1/D), D=1024).
BETA = 32.0 ** 0.5
ALPHA = 1.0 / BETA
LN8 = float(np.log(8.0))

# Problem constants (hardcoded per the harness contract).
B, T, D, V = 2, 2048, 1024, 50257
N_TOK = B * T
N_CORES = 8
VS = 6400                 # per-core padded vocab shard (8 * 6400 = 51200 >= V)
BIAS_PAD = -10000.0       # exp(x + BIAS_PAD) == 0 in fp32 for any real logit


def build_ce_kernel_c(n_tok, d_model, vs, n_gtiles, n_cores):
    """Variant c: host-pretransposed fp8 operands, PE-dense main loop."""
    n_dt = d_model // P        # K tiles of 128
    n_vt = vs // P             # vocab tiles of 128
    n_tb = n_tok // 512        # token blocks of 512 (s accumulator slots)
    n_tg = n_tok // 1024       # token groups of 1024 (PSUM mm granularity)
    assert n_dt % 2 == 0 and n_vt % 2 == 0 and n_tok % 1024 == 0

    nc = bacc.Bacc("TRN2", target_bir_lowering=False, debug=False,
                   num_devices=n_cores)

    # fp8 matmul operands, pre-transposed/scaled/padded on the host
    ht8_in = nc.dram_tensor("ht8", [d_model, n_tok], FP8, kind="ExternalInput")
    et8_in = nc.dram_tensor("et8", [d_model, vs], FP8, kind="ExternalInput")
    # per-partition exp bias: bias_pp[p, vt] = b[vt*128+p] - ln8 (or pad)
    bpp_in = nc.dram_tensor("bpp", [P, n_vt], F32, kind="ExternalInput")
    # fp32 copies for the target-logit gather path
    h_in = nc.dram_tensor("h", [n_tok, d_model], F32, kind="ExternalInput")
    e_in = nc.dram_tensor("e", [vs, d_model], F32, kind="ExternalInput")
    b_in = nc.dram_tensor("b", [vs], F32, kind="ExternalInput")
    gl_in = nc.dram_tensor("g_lbl", [n_gtiles, P], I32, kind="ExternalInput")
    gt_in = nc.dram_tensor("g_tok", [n_gtiles, P], I32, kind="ExternalInput")
    gm_in = nc.dram_tensor("g_mask", [n_gtiles, P], F32, kind="ExternalInput")
    loss_out = nc.dram_tensor("loss", [1, 1], F32, kind="ExternalOutput")

    # allreduce payload: n_tok sumexp/8 values + 128 tgt partials
    cc_len = n_tok + P
    cc_in = nc.dram_tensor("cc_in", [cc_len], F32)
    cc_out = nc.dram_tensor("cc_out", [cc_len], F32, addr_space="Shared")

    with tile.TileContext(nc, num_cores=n_cores) as tc:
        with ExitStack() as ctx:
            const = ctx.enter_context(tc.tile_pool(name="const", bufs=1))
            big_pool = ctx.enter_context(tc.tile_pool(name="big", bufs=1))
            exp_pool = ctx.enter_context(tc.tile_pool(name="expp", bufs=2))
            g_pool = ctx.enter_context(tc.tile_pool(name="g", bufs=6))
            fin_pool = ctx.enter_context(tc.tile_pool(name="fin", bufs=1))
            acc_pool = ctx.enter_context(tc.tile_pool(name="accp", bufs=1))
            mm_psum = ctx.enter_context(
                tc.tile_pool(name="mm_psum", bufs=2, space="PSUM"))

            # ---- constants ----
            ones1b = const.tile([P, 1], BF16)     # ones for final vocab sum
            nc.vector.memset(ones1b[:], 1.0)
            ones128 = const.tile([P, 1], F32)
            nc.vector.memset(ones128[:], 1.0)
            nones128 = const.tile([P, 1], F32)
            nc.vector.memset(nones128[:], -1.0)
            zbias = const.tile([P, 1], F32)
            nc.vector.memset(zbias[:], 0.0)
            bias_pp = const.tile([P, n_vt], F32)
            nc.sync.dma_start(bias_pp[:], bpp_in[:, :])

            # ---- fp8 operands straight from DRAM (no on-device prep) ----
            # layout [p, dc, x] with d = dc*128 + p; same factorization for
            # both operands so the contraction lines up.
            ht8 = big_pool.tile([P, n_dt, n_tok], FP8)
            ht8_src = ht8_in.rearrange("(dc p) t -> p dc t", p=P)
            tchunk = 1024
            for t0 in range(0, n_tok, tchunk):
                nc.sync.dma_start(ht8[:, :, t0:t0 + tchunk],
                                  ht8_src[:, :, t0:t0 + tchunk])
            et8 = big_pool.tile([P, n_dt, vs], FP8)
            et8_src = et8_in.rearrange("(dc p) v -> p dc v", p=P)
            vchunk = 640 if vs % 640 == 0 else P
            for v0 in range(0, vs, vchunk):
                nc.sync.dma_start(et8[:, :, v0:v0 + vchunk],
                                  et8_src[:, :, v0:v0 + vchunk])

            # ---- target logits (gather path; overlaps the main loop) ----
            lbl_sb = fin_pool.tile([P, n_gtiles], I32)
            tok_sb = fin_pool.tile([P, n_gtiles], I32)
            msk_sb = fin_pool.tile([P, n_gtiles], F32)
            nc.sync.dma_start(lbl_sb[:], gl_in.rearrange("g p -> p g"))
            nc.sync.dma_start(tok_sb[:], gt_in.rearrange("g p -> p g"))
            nc.sync.dma_start(msk_sb[:], gm_in.rearrange("g p -> p g"))
            dots = fin_pool.tile([P, n_gtiles], F32)
            bg = fin_pool.tile([P, n_gtiles], F32)
            for g in range(n_gtiles):
                eg = g_pool.tile([P, d_model], F32, tag="grow")
                nc.gpsimd.indirect_dma_start(
                    out=eg[:], out_offset=None, in_=e_in[:, :],
                    in_offset=bass.IndirectOffsetOnAxis(
                        ap=lbl_sb[:, g:g + 1], axis=0))
                hg = g_pool.tile([P, d_model], F32, tag="grow")
                nc.gpsimd.indirect_dma_start(
                    out=hg[:], out_offset=None, in_=h_in[:, :],
                    in_offset=bass.IndirectOffsetOnAxis(
                        ap=tok_sb[:, g:g + 1], axis=0))
                nc.gpsimd.indirect_dma_start(
                    out=bg[:, g:g + 1], out_offset=None,
                    in_=b_in.rearrange("(v o) -> v o", o=1),
                    in_offset=bass.IndirectOffsetOnAxis(
                        ap=lbl_sb[:, g:g + 1], axis=0))
                gsc = g_pool.tile([P, d_model], F32, tag="grow")
                nc.vector.tensor_mul(gsc[:], eg[:], hg[:])
                nc.vector.tensor_reduce(
                    dots[:, g:g + 1], gsc[:],
                    axis=mybir.AxisListType.X, op=ALU.add)
            dsum = fin_pool.tile([P, n_gtiles], F32)
            nc.vector.tensor_add(dsum[:], dots[:], bg[:])
            dmask = fin_pool.tile([P, n_gtiles], F32)
            nc.vector.tensor_mul(dmask[:], dsum[:], msk_sb[:])
            tgt_red = fin_pool.tile([P, 1], F32)
            nc.vector.tensor_reduce(
                tgt_red[:], dmask[:], axis=mybir.AxisListType.X, op=ALU.add)

            # ---- main loop: token super-groups outer, vocab inner ----
            # Both fp8 operands are SBUF-resident. Per super-group (TS
            # tokens = one 4-bank PSUM tile) the PE streams 100%
            # homogeneous fp8 DoubleRow matmuls; ScalarE does one wide
            # exp per vocab tile; the DVE accumulates exp tiles into an
            # SBUF f32 accumulator (vocab partition-sums happen once, at
            # the tail, via cheap bf16 ones-matmuls).
            TS = 2048 if n_tok % 2048 == 0 else n_tok
            n_ts = n_tok // TS
            nk = TS // 512
            accs = []
            for ts in range(n_ts):
                acc = acc_pool.tile([P, TS], F32, name=f"acc{ts}")
                accs.append(acc)
                for vt in range(n_vt):
                    ps = mm_psum.tile([P, TS], F32, tag="mm",
                                      name=f"ps{ts}_{vt}")
                    for j in range(n_dt // 2):
                        for k in range(nk):
                            nc.tensor.matmul(
                                ps[:, k * 512:(k + 1) * 512],
                                lhsT=et8[:, 2 * j:2 * j + 2,
                                         vt * P:(vt + 1) * P],
                                rhs=ht8[:, 2 * j:2 * j + 2,
                                        ts * TS + k * 512:
                                        ts * TS + (k + 1) * 512],
                                start=(j == 0), stop=(j == n_dt // 2 - 1),
                                perf_mode=DR)
                    ex = exp_pool.tile([P, TS], BF16, tag="exp",
                                       name=f"exp{ts}_{vt}")
                    nc.scalar.activation(
                        ex[:], ps[:], AF.Exp, bias=bias_pp[:, vt:vt + 1])
                    if vt == 0:
                        nc.vector.tensor_copy(acc[:], ex[:])
                    else:
                        nc.vector.tensor_add(acc[:], acc[:], ex[:])

            # ---- tail: vocab partition-sums + ship partials ----
            s_sbs = [fin_pool.tile([P, 512], F32, name=f"s_sb{i}")
                     for i in range(-(-n_tb // 4))]

            def s_row(tb):
                return s_sbs[tb // 4][32 * (tb % 4):32 * (tb % 4) + 1, :]

            for ts in range(n_ts):
                accb = exp_pool.tile([P, TS], BF16, tag="exp",
                                     name=f"accb{ts}")
                nc.vector.tensor_copy(accb[:], accs[ts][:])
                red = mm_psum.tile([P, TS], F32, tag="mm", name=f"red{ts}")
                for k in range(nk):
                    tb = ts * nk + k
                    nc.tensor.matmul(
                        red[0:1, k * 512:(k + 1) * 512], lhsT=ones1b[:],
                        rhs=accb[:, k * 512:(k + 1) * 512],
                        start=True, stop=True, skip_group_check=True)
                for k in range(nk):
                    tb = ts * nk + k
                    nc.vector.tensor_copy(
                        s_row(tb), red[0:1, k * 512:(k + 1) * 512])
                    nc.sync.dma_start(
                        cc_in[tb * 512:(tb + 1) * 512].rearrange(
                            "(a b) -> a b", a=1),
                        s_row(tb))

            # ---- allreduce S partials + tgt partials ----
            nc.sync.dma_start(
                cc_in[n_tok:cc_len].rearrange("(a b) -> a b", a=P),
                tgt_red[:])
            nc.gpsimd.collective_compute(
                "AllReduce", ALU.add,
                replica_groups=[list(range(n_cores))],
                ins=[cc_in.rearrange("(a b) -> a b", a=8)],
                outs=[cc_out.rearrange("(a b) -> a b", a=8)])
            s_glob = fin_pool.tile([n_tb, 512], F32)
            nc.sync.dma_start(
                s_glob[:], cc_out[0:n_tok].rearrange("(a b) -> a b", a=n_tb))
            tgt_glob = fin_pool.tile([P, 1], F32)
            nc.sync.dma_start(
                tgt_glob[:],
                cc_out[n_tok:cc_len].rearrange("(a b) -> a b", a=P))

            # ---- loss = mean(ln S') + ln 8 - mean(tgt) ----
            lse = fin_pool.tile([n_tb, 512], F32)
            lse_sum = fin_pool.tile([n_tb, 1], F32)
            nc.scalar.activation(
                lse[:], s_glob[:], AF.Ln, bias=zbias[0:n_tb, :],
                accum_out=lse_sum[:])
            lp = mm_psum.tile([1, 1], F32, tag="mm")
            nc.tensor.matmul(lp[:], lhsT=ones128[0:n_tb, :],
                             rhs=lse_sum[:], start=True, stop=False,
                             skip_group_check=True)
            nc.tensor.matmul(lp[:], lhsT=nones128[:], rhs=tgt_glob[:],
                             start=False, stop=True, skip_group_check=True)
            loss_sb = fin_pool.tile([1, 1], F32)
            nc.scalar.activation(loss_sb[:], lp[:], AF.Copy,
                                 scale=1.0 / float(n_tok), bias=LN8)
            nc.sync.dma_start(loss_out[:, :], loss_sb[:])

    nc.finalize()
    return nc


def host_prepare(outputs, word_embeddings, word_biases, labels,
                 n_cores=N_CORES, vs=None):
    """Shard/pad/transpose/quantize the full inputs into per-core maps."""
    d_model = outputs.shape[-1]
    v_real = word_embeddings.shape[0]
    n_tok = outputs.shape[0] * outputs.shape[1]
    if vs is None:
        vs = -(-v_real // (n_cores * 2 * P)) * 2 * P  # per-core, mult of 256
    v_pad = n_cores * vs
    n_vt = vs // P

    h = np.ascontiguousarray(
        np.asarray(outputs, dtype=np.float32).reshape(n_tok, d_model))
    e_pad = np.zeros((v_pad, d_model), dtype=np.float32)
    e_pad[:v_real] = np.asarray(word_embeddings, dtype=np.float32)
    b_pad = np.full(v_pad, BIAS_PAD, dtype=np.float32)
    b_pad[:v_real] = np.asarray(word_biases, dtype=np.float32)
    lab = np.asarray(labels).reshape(-1).astype(np.int64)

    fp8 = ml_dtypes.float8_e4m3
    ht8 = np.ascontiguousarray((h.T * ALPHA)).astype(fp8)

    # Per-core gather lists: labels that fall inside each core's shard.
    sels = [np.nonzero((lab >= c * vs) & (lab < (c + 1) * vs))[0]
            for c in range(n_cores)]
    cap = max(max((len(s) for s in sels), default=1), 1)
    n_gtiles = -(-cap // P)
    gcap = n_gtiles * P

    in_maps = []
    for c in range(n_cores):
        sel = sels[c]
        g_lbl = np.zeros(gcap, dtype=np.int32)
        g_tok = np.zeros(gcap, dtype=np.int32)
        g_msk = np.zeros(gcap, dtype=np.float32)
        g_lbl[:len(sel)] = (lab[sel] - c * vs).astype(np.int32)
        g_tok[:len(sel)] = sel.astype(np.int32)
        g_msk[:len(sel)] = 1.0
        e_c = e_pad[c * vs:(c + 1) * vs]
        b_c = b_pad[c * vs:(c + 1) * vs]
        # exp bias per vocab-partition, with the -ln8 overflow guard; the
        # pad bias stays hugely negative so exp() of pad vocab is 0.
        bpp = b_c.reshape(n_vt, P).T.copy()
        bpp[bpp > BIAS_PAD / 2] -= LN8
        in_maps.append({
            "ht8": ht8,
            "et8": np.ascontiguousarray(e_c.T * BETA).astype(fp8),
            "bpp": np.ascontiguousarray(bpp),
            "h": h,
            "e": np.ascontiguousarray(e_c),
            "b": np.ascontiguousarray(b_c),
            "g_lbl": g_lbl.reshape(n_gtiles, P),
            "g_tok": g_tok.reshape(n_gtiles, P),
            "g_mask": g_msk.reshape(n_gtiles, P),
        })
    meta = dict(n_tok=n_tok, d_model=d_model, vs=vs, n_gtiles=n_gtiles,
                n_cores=n_cores)
    return in_maps, meta


_KERNEL_CACHE = {}
VARIANT = "c"


def _get_kernel(meta, variant=None):
    if variant is None:
        variant = VARIANT
    key = tuple(sorted(meta.items())) + (variant,)
    if key not in _KERNEL_CACHE:
        _KERNEL_CACHE[key] = build_ce_kernel_c(**meta)
    return _KERNEL_CACHE[key]


def kernel(outputs, word_embeddings, word_biases, labels):
    from concourse.bass_utils import run_bass_kernel_spmd

    in_maps, meta = host_prepare(outputs, word_embeddings, word_biases,
                                 labels, n_cores=N_CORES, vs=VS)
    nc = _get_kernel(meta)
    res = run_bass_kernel_spmd(nc, in_maps, list(range(meta["n_cores"])))
    loss = res.results[0]["loss"][0, 0]
    return np.float32(loss)



# revision 3
# speedup vs baseline: 1.0090x; 1.0090x over previous
"""Distributed cross-entropy loss kernel for Trainium2 (8 NeuronCores).

loss = -mean_t(log_softmax(h @ E^T + b)[t, labels[t]])
     = mean_t(LSE_t) - mean_t(h_t . E[labels[t]] + b[labels[t]])

Strategy (variant c): vocab tensor parallel across 8 cores; the host
pre-transposes and pre-quantizes both matmul operands to fp8e4 so the
device does nothing but fp8 DoubleRow matmuls, per-partition-biased
exp on the scalar engine, and fp8 DoubleRow ones-matmul partition
reductions. A tiny AllReduce combines per-core sumexp/target partials
and every core finishes the log + mean locally.

Orientation: vocab on PSUM partitions, tokens moving. Per (vocab-tile,
token-group) the PE accumulates K=1024 in 4 DoubleRow passes, ScalarE
applies exp(logit + b_v - ln8) writing fp8, and pairs of vocab tiles
are reduced over partitions with a DoubleRow ones-matmul into 8
persistent [1,512] PSUM accumulators (3 banks, offsets 0/32/64).

exp is pre-scaled by 1/8 (bias shift of -ln 8) so values fit e4m3's
max of 448; the final loss adds ln 8 back.
"""

from contextlib import ExitStack

import numpy as np
import ml_dtypes

import concourse.bass as bass
import concourse.tile as tile
from concourse import bacc, mybir

F32 = mybir.dt.float32
BF16 = mybir.dt.bfloat16
FP8 = mybir.dt.float8e4
I32 = mybir.dt.int32
AF = mybir.ActivationFunctionType
ALU = mybir.AluOpType
DR = mybir.MatmulPerfMode.DoubleRow

P = 128

# fp8 operand scaling: h' = ALPHA*h, E' = BETA*E with ALPHA*BETA == 1, so
# logits keep their true scale. Balancing puts both operands at ~0.18 std,
# inside e4m3's normal range (h ~ N(0,1), E rows ~ N(0, 1/D), D=1024).
BETA = 32.0 ** 0.5
ALPHA = 1.0 / BETA
LN8 = float(np.log(8.0))

# Problem constants (hardcoded per the harness contract).
B, T, D, V = 2, 2048, 1024, 50257
N_TOK = B * T
N_CORES = 8
VS = 6400                 # per-core padded vocab shard (8 * 6400 = 51200 >= V)
BIAS_PAD = -10000.0       # exp(x + BIAS_PAD) == 0 in fp32 for any real logit


def build_ce_kernel_c(n_tok, d_model, vs, n_gtiles, n_cores):
    """Variant c: host-pretransposed fp8 operands, PE-dense main loop."""
    n_dt = d_model // P        # K tiles of 128
    n_vt = vs // P             # vocab tiles of 128
    n_tb = n_tok // 512        # token blocks of 512 (s accumulator slots)
    n_tg = n_tok // 1024       # token groups of 1024 (PSUM mm granularity)
    assert n_dt % 2 == 0 and n_vt % 2 == 0 and n_tok % 1024 == 0

    nc = bacc.Bacc("TRN2", target_bir_lowering=False, debug=False,
                   num_devices=n_cores)

    # fp8 matmul operands, pre-transposed/scaled/padded on the host
    ht8_in = nc.dram_tensor("ht8", [d_model, n_tok], FP8, kind="ExternalInput")
    et8_in = nc.dram_tensor("et8", [d_model, vs], FP8, kind="ExternalInput")
    # per-partition exp bias: bias_pp[p, vt] = b[vt*128+p] - ln8 (or pad)
    bpp_in = nc.dram_tensor("bpp", [P, n_vt], F32, kind="ExternalInput")
    # fp32 copies for the target-logit gather path
    h_in = nc.dram_tensor("h", [n_tok, d_model], F32, kind="ExternalInput")
    e_in = nc.dram_tensor("e", [vs, d_model], F32, kind="ExternalInput")
    b_in = nc.dram_tensor("b", [vs], F32, kind="ExternalInput")
    gl_in = nc.dram_tensor("g_lbl", [n_gtiles, P], I32, kind="ExternalInput")
    gt_in = nc.dram_tensor("g_tok", [n_gtiles, P], I32, kind="ExternalInput")
    gm_in = nc.dram_tensor("g_mask", [n_gtiles, P], F32, kind="ExternalInput")
    loss_out = nc.dram_tensor("loss", [1, 1], F32, kind="ExternalOutput")

    # allreduce payload: n_tok sumexp/8 values + 128 tgt partials
    cc_len = n_tok + P
    cc_in = nc.dram_tensor("cc_in", [cc_len], F32)
    cc_out = nc.dram_tensor("cc_out", [cc_len], F32, addr_space="Shared")

    with tile.TileContext(nc, num_cores=n_cores) as tc:
        with ExitStack() as ctx:
            const = ctx.enter_context(tc.tile_pool(name="const", bufs=1))
            big_pool = ctx.enter_context(tc.tile_pool(name="big", bufs=1))
            exp_pool = ctx.enter_context(tc.tile_pool(name="expp", bufs=2))
            g_pool = ctx.enter_context(tc.tile_pool(name="g", bufs=6))
            fin_pool = ctx.enter_context(tc.tile_pool(name="fin", bufs=1))
            acc_pool = ctx.enter_context(tc.tile_pool(name="accp", bufs=1))
            mm_psum = ctx.enter_context(
                tc.tile_pool(name="mm_psum", bufs=2, space="PSUM"))

            # ---- constants ----
            ones1b = const.tile([P, 1], BF16)     # ones for final vocab sum
            nc.vector.memset(ones1b[:], 1.0)
            ones128 = const.tile([P, 1], F32)
            nc.vector.memset(ones128[:], 1.0)
            nones128 = const.tile([P, 1], F32)
            nc.vector.memset(nones128[:], -1.0)
            zbias = const.tile([P, 1], F32)
            nc.vector.memset(zbias[:], 0.0)
            bias_pp = const.tile([P, n_vt], F32)
            nc.sync.dma_start(bias_pp[:], bpp_in[:, :])

            # ---- fp8 operands straight from DRAM (no on-device prep) ----
            # layout [p, dc, x] with d = dc*128 + p; same factorization for
            # both operands so the contraction lines up.
            ht8 = big_pool.tile([P, n_dt, n_tok], FP8)
            ht8_src = ht8_in.rearrange("(dc p) t -> p dc t", p=P)
            tchunk = 1024
            for t0 in range(0, n_tok, tchunk):
                nc.sync.dma_start(ht8[:, :, t0:t0 + tchunk],
                                  ht8_src[:, :, t0:t0 + tchunk])
            et8 = big_pool.tile([P, n_dt, vs], FP8)
            et8_src = et8_in.rearrange("(dc p) v -> p dc v", p=P)
            vchunk = 640 if vs % 640 == 0 else P
            for v0 in range(0, vs, vchunk):
                nc.sync.dma_start(et8[:, :, v0:v0 + vchunk],
                                  et8_src[:, :, v0:v0 + vchunk])

            # ---- target logits (gather path; overlaps the main loop) ----
            lbl_sb = fin_pool.tile([P, n_gtiles], I32)
            tok_sb = fin_pool.tile([P, n_gtiles], I32)
            msk_sb = fin_pool.tile([P, n_gtiles], F32)
            nc.sync.dma_start(lbl_sb[:], gl_in.rearrange("g p -> p g"))
            nc.sync.dma_start(tok_sb[:], gt_in.rearrange("g p -> p g"))
            nc.sync.dma_start(msk_sb[:], gm_in.rearrange("g p -> p g"))
            dots = fin_pool.tile([P, n_gtiles], F32)
            bg = fin_pool.tile([P, n_gtiles], F32)
            for g in range(n_gtiles):
                eg = g_pool.tile([P, d_model], F32, tag="grow")
                nc.gpsimd.indirect_dma_start(
                    out=eg[:], out_offset=None, in_=e_in[:, :],
                    in_offset=bass.IndirectOffsetOnAxis(
                        ap=lbl_sb[:, g:g + 1], axis=0))
                hg = g_pool.tile([P, d_model], F32, tag="grow")
                nc.gpsimd.indirect_dma_start(
                    out=hg[:], out_offset=None, in_=h_in[:, :],
                    in_offset=bass.IndirectOffsetOnAxis(
                        ap=tok_sb[:, g:g + 1], axis=0))
                nc.gpsimd.indirect_dma_start(
                    out=bg[:, g:g + 1], out_offset=None,
                    in_=b_in.rearrange("(v o) -> v o", o=1),
                    in_offset=bass.IndirectOffsetOnAxis(
                        ap=lbl_sb[:, g:g + 1], axis=0))
                gsc = g_pool.tile([P, d_model], F32, tag="grow")
                nc.vector.tensor_mul(gsc[:], eg[:], hg[:])
                nc.vector.tensor_reduce(
                    dots[:, g:g + 1], gsc[:],
                    axis=mybir.AxisListType.X, op=ALU.add)
            dsum = fin_pool.tile([P, n_gtiles], F32)
            nc.vector.tensor_add(dsum[:], dots[:], bg[:])
            dmask = fin_pool.tile([P, n_gtiles], F32)
            nc.vector.tensor_mul(dmask[:], dsum[:], msk_sb[:])
            tgt_red = fin_pool.tile([P, 1], F32)
            nc.vector.tensor_reduce(
                tgt_red[:], dmask[:], axis=mybir.AxisListType.X, op=ALU.add)

            # ---- main loop: token super-groups outer, vocab inner ----
            # Both fp8 operands are SBUF-resident. Per super-group (TS
            # tokens = one 4-bank PSUM tile) the PE streams 100%
            # homogeneous fp8 DoubleRow matmuls; ScalarE does one wide
            # exp per vocab tile; the DVE accumulates exp tiles into an
            # SBUF f32 accumulator (vocab partition-sums happen once, at
            # the tail, via cheap bf16 ones-matmuls).
            TS = 2048 if n_tok % 2048 == 0 else n_tok
            n_ts = n_tok // TS
            nk = TS // 512
            accs = []
            for ts in range(n_ts):
                acc = acc_pool.tile([P, TS], F32, name=f"acc{ts}")
                accs.append(acc)
                for vt in range(n_vt):
                    ps = mm_psum.tile([P, TS], F32, tag="mm",
                                      name=f"ps{ts}_{vt}")
                    for j in range(n_dt // 2):
                        for k in range(nk):
                            nc.tensor.matmul(
                                ps[:, k * 512:(k + 1) * 512],
                                lhsT=et8[:, 2 * j:2 * j + 2,
                                         vt * P:(vt + 1) * P],
                                rhs=ht8[:, 2 * j:2 * j + 2,
                                        ts * TS + k * 512:
                                        ts * TS + (k + 1) * 512],
                                start=(j == 0), stop=(j == n_dt // 2 - 1),
                                perf_mode=DR)
                    ex = exp_pool.tile([P, TS], BF16, tag="exp",
                                       name=f"exp{ts}_{vt}")
                    nc.scalar.activation(
                        ex[:], ps[:], AF.Exp, bias=bias_pp[:, vt:vt + 1])
                    if vt == 0:
                        nc.vector.tensor_copy(acc[:], ex[:])
                    else:
                        nc.vector.tensor_add(acc[:], acc[:], ex[:])

            # ---- tail: vocab partition-sums + ship partials ----
            s_sbs = [fin_pool.tile([P, 512], F32, name=f"s_sb{i}")
                     for i in range(-(-n_tb // 4))]

            def s_row(tb):
                return s_sbs[tb // 4][32 * (tb % 4):32 * (tb % 4) + 1, :]

            for ts in range(n_ts):
                accb = exp_pool.tile([P, TS], BF16, tag="exp",
                                     name=f"accb{ts}")
                nc.vector.tensor_copy(accb[:], accs[ts][:])
                red = mm_psum.tile([P, TS], F32, tag="mm", name=f"red{ts}")
                for k in range(nk):
                    tb = ts * nk + k
                    nc.tensor.matmul(
                        red[0:1, k * 512:(k + 1) * 512], lhsT=ones1b[:],
                        rhs=accb[:, k * 512:(k + 1) * 512],
                        start=True, stop=True, skip_group_check=True)
                for k in range(nk):
                    tb = ts * nk + k
                    nc.vector.tensor_copy(
                        s_row(tb), red[0:1, k * 512:(k + 1) * 512])
                    nc.sync.dma_start(
                        cc_in[tb * 512:(tb + 1) * 512].rearrange(
                            "(a b) -> a b", a=1),
                        s_row(tb))

            # ---- allreduce S partials + tgt partials ----
            nc.sync.dma_start(
                cc_in[n_tok:cc_len].rearrange("(a b) -> a b", a=P),
                tgt_red[:])
            nc.gpsimd.collective_compute(
                "AllReduce", ALU.add,
                replica_groups=[list(range(n_cores))],
                ins=[cc_in.rearrange("(a b) -> a b", a=8)],
                outs=[cc_out.rearrange("(a b) -> a b", a=8)])
            s_glob = fin_pool.tile([n_tb, 512], F32)
            nc.sync.dma_start(
                s_glob[:], cc_out[0:n_tok].rearrange("(a b) -> a b", a=n_tb))
            tgt_glob = fin_pool.tile([P, 1], F32)
            nc.sync.dma_start(
                tgt_glob[:],
                cc_out[n_tok:cc_len].rearrange("(a b) -> a b", a=P))

            # ---- loss = mean(ln S') + ln 8 - mean(tgt) ----
            lse = fin_pool.tile([n_tb, 512], F32)
            lse_sum = fin_pool.tile([n_tb, 1], F32)
            nc.scalar.activation(
                lse[:], s_glob[:], AF.Ln, bias=zbias[0:n_tb, :],
                accum_out=lse_sum[:])
            lp = mm_psum.tile([1, 1], F32, tag="mm")
            nc.tensor.matmul(lp[:], lhsT=ones128[0:n_tb, :],
                             rhs=lse_sum[:], start=True, stop=False,
                             skip_group_check=True)
            nc.tensor.matmul(lp[:], lhsT=nones128[:], rhs=tgt_glob[:],
                             start=False, stop=True, skip_group_check=True)
            loss_sb = fin_pool.tile([1, 1], F32)
            nc.scalar.activation(loss_sb[:], lp[:], AF.Copy,
                                 scale=1.0 / float(n_tok), bias=LN8)
            nc.sync.dma_start(loss_out[:, :], loss_sb[:])

    nc.finalize()
    return nc


def build_ce_kernel_d(n_tok, d_model, vs, n_gtiles, n_cores):
    """Variant d: variant c plus
    - chunked et8/ht8 input tiles so the PE starts ~8us in (not ~30us)
    - bf16 accumulator (DVE 2x_1p mode, and no tail copy to bf16)
    - per-ts split AllReduce: ts0's collective (+ target-logit payload)
      overlaps ts1 compute; only ts1's small collective is exposed
    """
    n_dt = d_model // P        # K tiles of 128
    n_vt = vs // P             # vocab tiles of 128
    n_tb = n_tok // 512        # token blocks of 512
    assert n_dt % 2 == 0 and n_tok % 1024 == 0

    TS = 2048 if n_tok % 2048 == 0 else n_tok
    n_ts = n_tok // TS
    nk = TS // 512

    # input chunk sizes
    TOK_CH = 1024
    n_ht = n_tok // TOK_CH
    CH_V = 640 if vs % 640 == 0 else vs
    n_ec = vs // CH_V
    ch_vt = CH_V // P          # vocab tiles per et8 chunk

    nc = bacc.Bacc("TRN2", target_bir_lowering=False, debug=False,
                   num_devices=n_cores)

    ht8_in = nc.dram_tensor("ht8", [d_model, n_tok], FP8, kind="ExternalInput")
    et8_in = nc.dram_tensor("et8", [d_model, vs], FP8, kind="ExternalInput")
    bpp_in = nc.dram_tensor("bpp", [P, n_vt], F32, kind="ExternalInput")
    h_in = nc.dram_tensor("h", [n_tok, d_model], F32, kind="ExternalInput")
    e_in = nc.dram_tensor("e", [vs, d_model], F32, kind="ExternalInput")
    b_in = nc.dram_tensor("b", [vs], F32, kind="ExternalInput")
    gl_in = nc.dram_tensor("g_lbl", [n_gtiles, P], I32, kind="ExternalInput")
    gt_in = nc.dram_tensor("g_tok", [n_gtiles, P], I32, kind="ExternalInput")
    gm_in = nc.dram_tensor("g_mask", [n_gtiles, P], F32, kind="ExternalInput")
    loss_out = nc.dram_tensor("loss", [1, 1], F32, kind="ExternalOutput")

    # collective payloads: cc0 = ts0 sumexp + tgt partials; cc1 = rest
    cc0_len = TS + P
    cc0_in = nc.dram_tensor("cc0_in", [cc0_len], F32)
    cc0_out = nc.dram_tensor("cc0_out", [cc0_len], F32, addr_space="Shared")
    if n_ts > 1:
        cc1_len = (n_ts - 1) * TS
        cc1_in = nc.dram_tensor("cc1_in", [cc1_len], F32)
        cc1_out = nc.dram_tensor("cc1_out", [cc1_len], F32,
                                 addr_space="Shared")

    with tile.TileContext(nc, num_cores=n_cores) as tc:
        with ExitStack() as ctx:
            const = ctx.enter_context(tc.tile_pool(name="const", bufs=1))
            big_pool = ctx.enter_context(tc.tile_pool(name="big", bufs=1))
            exp_pool = ctx.enter_context(tc.tile_pool(name="expp", bufs=2))
            g_pool = ctx.enter_context(tc.tile_pool(name="g", bufs=6))
            fin_pool = ctx.enter_context(tc.tile_pool(name="fin", bufs=1))
            acc_pool = ctx.enter_context(tc.tile_pool(name="accp", bufs=1))
            mm_psum = ctx.enter_context(
                tc.tile_pool(name="mm_psum", bufs=2, space="PSUM"))

            # ---- constants ----
            ones1b = const.tile([P, 1], BF16)
            nc.vector.memset(ones1b[:], 1.0)
            ones128 = const.tile([P, 1], F32)
            nc.vector.memset(ones128[:], 1.0)
            nones128 = const.tile([P, 1], F32)
            nc.vector.memset(nones128[:], -1.0)
            zbias = const.tile([P, 1], F32)
            nc.vector.memset(zbias[:], 0.0)
            bias_pp = const.tile([P, n_vt], F32)
            nc.sync.dma_start(bias_pp[:], bpp_in[:, :])

            # ---- chunked fp8 operand loads, ordered for earliest start ----
            ht8_src = ht8_in.rearrange("(dc p) t -> p dc t", p=P)
            et8_src = et8_in.rearrange("(dc p) v -> p dc v", p=P)
            et_c = [big_pool.tile([P, n_dt, CH_V], FP8, name=f"et{c}")
                    for c in range(n_ec)]
            ht_t = [big_pool.tile([P, n_dt, TOK_CH], FP8, name=f"ht{i}")
                    for i in range(n_ht)]
            nc.sync.dma_start(et_c[0][:],
                              et8_src[:, :, 0:CH_V])
            for i in range(min(2, n_ht)):
                nc.sync.dma_start(ht_t[i][:],
                                  ht8_src[:, :, i * TOK_CH:(i + 1) * TOK_CH])
            # gather-path metadata (tiny; lets gpsimd start early)
            lbl_sb = fin_pool.tile([P, n_gtiles], I32)
            tok_sb = fin_pool.tile([P, n_gtiles], I32)
            msk_sb = fin_pool.tile([P, n_gtiles], F32)
            nc.sync.dma_start(lbl_sb[:], gl_in.rearrange("g p -> p g"))
            nc.sync.dma_start(tok_sb[:], gt_in.rearrange("g p -> p g"))
            nc.sync.dma_start(msk_sb[:], gm_in.rearrange("g p -> p g"))
            for c in range(1, n_ec):
                nc.sync.dma_start(et_c[c][:],
                                  et8_src[:, :, c * CH_V:(c + 1) * CH_V])
            for i in range(2, n_ht):
                nc.sync.dma_start(ht_t[i][:],
                                  ht8_src[:, :, i * TOK_CH:(i + 1) * TOK_CH])

            # ---- target logits (gather path; overlaps the main loop) ----
            dots = fin_pool.tile([P, n_gtiles], F32)
            bg = fin_pool.tile([P, n_gtiles], F32)
            for g in range(n_gtiles):
                eg = g_pool.tile([P, d_model], F32, tag="grow")
                nc.gpsimd.indirect_dma_start(
                    out=eg[:], out_offset=None, in_=e_in[:, :],
                    in_offset=bass.IndirectOffsetOnAxis(
                        ap=lbl_sb[:, g:g + 1], axis=0))
                hg = g_pool.tile([P, d_model], F32, tag="grow")
                nc.gpsimd.indirect_dma_start(
                    out=hg[:], out_offset=None, in_=h_in[:, :],
                    in_offset=bass.IndirectOffsetOnAxis(
                        ap=tok_sb[:, g:g + 1], axis=0))
                nc.gpsimd.indirect_dma_start(
                    out=bg[:, g:g + 1], out_offset=None,
                    in_=b_in.rearrange("(v o) -> v o", o=1),
                    in_offset=bass.IndirectOffsetOnAxis(
                        ap=lbl_sb[:, g:g + 1], axis=0))
                gsc = g_pool.tile([P, d_model], F32, tag="grow")
                nc.vector.tensor_mul(gsc[:], eg[:], hg[:])
                nc.vector.tensor_reduce(
                    dots[:, g:g + 1], gsc[:],
                    axis=mybir.AxisListType.X, op=ALU.add)
            dsum = fin_pool.tile([P, n_gtiles], F32)
            nc.vector.tensor_add(dsum[:], dots[:], bg[:])
            dmask = fin_pool.tile([P, n_gtiles], F32)
            nc.vector.tensor_mul(dmask[:], dsum[:], msk_sb[:])
            tgt_red = fin_pool.tile([P, 1], F32)
            nc.vector.tensor_reduce(
                tgt_red[:], dmask[:], axis=mybir.AxisListType.X, op=ALU.add)

            # ---- final-reduction tiles ----
            s_glob = fin_pool.tile([n_tb, 512], F32)
            tgt_glob = fin_pool.tile([P, 1], F32)
            s_sbs = [fin_pool.tile([1, TS], F32, name=f"s_sb{t}")
                     for t in range(n_ts)]

            # ---- main loop ----
            for ts in range(n_ts):
                acc = acc_pool.tile([P, TS], BF16, name=f"acc{ts}")
                for vt in range(n_vt):
                    ec, vl = vt // ch_vt, vt % ch_vt
                    ps = mm_psum.tile([P, TS], F32, tag="mm",
                                      name=f"ps{ts}_{vt}")
                    for j in range(n_dt // 2):
                        for k in range(nk):
                            g0 = ts * TS + k * 512
                            ti, toff = g0 // TOK_CH, g0 % TOK_CH
                            nc.tensor.matmul(
                                ps[:, k * 512:(k + 1) * 512],
                                lhsT=et_c[ec][:, 2 * j:2 * j + 2,
                                              vl * P:(vl + 1) * P],
                                rhs=ht_t[ti][:, 2 * j:2 * j + 2,
                                             toff:toff + 512],
                                start=(j == 0), stop=(j == n_dt // 2 - 1),
                                perf_mode=DR)
                    if vt == 0:
                        nc.scalar.activation(
                            acc[:], ps[:], AF.Exp, bias=bias_pp[:, 0:1])
                    else:
                        ex = exp_pool.tile([P, TS], BF16, tag="exp",
                                           name=f"exp{ts}_{vt}")
                        nc.scalar.activation(
                            ex[:], ps[:], AF.Exp, bias=bias_pp[:, vt:vt + 1])
                        nc.vector.tensor_add(acc[:], acc[:], ex[:])

                # ---- ts tail: vocab partition-sum, ship, collective ----
                red = mm_psum.tile([P, TS], F32, tag="mm", name=f"red{ts}")
                for k in range(nk):
                    nc.tensor.matmul(
                        red[0:1, k * 512:(k + 1) * 512], lhsT=ones1b[:],
                        rhs=acc[:, k * 512:(k + 1) * 512],
                        start=True, stop=True, skip_group_check=True)
                nc.vector.tensor_copy(s_sbs[ts][:], red[0:1, :])
                if ts == 0:
                    nc.sync.dma_start(
                        cc0_in[0:TS].rearrange("(a b) -> a b", a=1),
                        s_sbs[0][:])
                    nc.sync.dma_start(
                        cc0_in[TS:cc0_len].rearrange("(a b) -> a b", a=P),
                        tgt_red[:])
                    nc.gpsimd.collective_compute(
                        "AllReduce", ALU.add,
                        replica_groups=[list(range(n_cores))],
                        ins=[cc0_in.rearrange("(a b) -> a b", a=8)],
                        outs=[cc0_out.rearrange("(a b) -> a b", a=8)])
                    nc.sync.dma_start(
                        s_glob[0:nk, :],
                        cc0_out[0:TS].rearrange("(a b) -> a b", a=nk))
                    nc.sync.dma_start(
                        tgt_glob[:],
                        cc0_out[TS:cc0_len].rearrange("(a b) -> a b", a=P))
                else:
                    o = (ts - 1) * TS
                    nc.sync.dma_start(
                        cc1_in[o:o + TS].rearrange("(a b) -> a b", a=1),
                        s_sbs[ts][:])

            if n_ts > 1:
                nc.gpsimd.collective_compute(
                    "AllReduce", ALU.add,
                    replica_groups=[list(range(n_cores))],
                    ins=[cc1_in.rearrange("(a b) -> a b", a=8)],
                    outs=[cc1_out.rearrange("(a b) -> a b", a=8)])
                nc.sync.dma_start(
                    s_glob[nk:n_tb, :],
                    cc1_out[:].rearrange("(a b) -> a b", a=n_tb - nk))

            # ---- loss = mean(ln S') + ln 8 - mean(tgt) ----
            lse = fin_pool.tile([n_tb, 512], F32)
            lse_sum = fin_pool.tile([n_tb, 1], F32)
            nc.scalar.activation(
                lse[:], s_glob[:], AF.Ln, bias=zbias[0:n_tb, :],
                accum_out=lse_sum[:])
            lp = mm_psum.tile([1, 1], F32, tag="mm")
            nc.tensor.matmul(lp[:], lhsT=ones128[0:n_tb, :],
                             rhs=lse_sum[:], start=True, stop=False,
                             skip_group_check=True)
            nc.tensor.matmul(lp[:], lhsT=nones128[:], rhs=tgt_glob[:],
                             start=False, stop=True, skip_group_check=True)
            loss_sb = fin_pool.tile([1, 1], F32)
            nc.scalar.activation(loss_sb[:], lp[:], AF.Copy,
                                 scale=1.0 / float(n_tok), bias=LN8)
            nc.sync.dma_start(loss_out[:, :], loss_sb[:])

    nc.finalize()
    return nc


def host_prepare(outputs, word_embeddings, word_biases, labels,
                 n_cores=N_CORES, vs=None):
    """Shard/pad/transpose/quantize the full inputs into per-core maps."""
    d_model = outputs.shape[-1]
    v_real = word_embeddings.shape[0]
    n_tok = outputs.shape[0] * outputs.shape[1]
    if vs is None:
        vs = -(-v_real // (n_cores * 2 * P)) * 2 * P  # per-core, mult of 256
    v_pad = n_cores * vs
    n_vt = vs // P

    h = np.ascontiguousarray(
        np.asarray(outputs, dtype=np.float32).reshape(n_tok, d_model))
    e_pad = np.zeros((v_pad, d_model), dtype=np.float32)
    e_pad[:v_real] = np.asarray(word_embeddings, dtype=np.float32)
    b_pad = np.full(v_pad, BIAS_PAD, dtype=np.float32)
    b_pad[:v_real] = np.asarray(word_biases, dtype=np.float32)
    lab = np.asarray(labels).reshape(-1).astype(np.int64)

    fp8 = ml_dtypes.float8_e4m3
    ht8 = np.ascontiguousarray((h.T * ALPHA)).astype(fp8)

    # Per-core gather lists: labels that fall inside each core's shard.
    sels = [np.nonzero((lab >= c * vs) & (lab < (c + 1) * vs))[0]
            for c in range(n_cores)]
    cap = max(max((len(s) for s in sels), default=1), 1)
    n_gtiles = -(-cap // P)
    gcap = n_gtiles * P

    in_maps = []
    for c in range(n_cores):
        sel = sels[c]
        g_lbl = np.zeros(gcap, dtype=np.int32)
        g_tok = np.zeros(gcap, dtype=np.int32)
        g_msk = np.zeros(gcap, dtype=np.float32)
        g_lbl[:len(sel)] = (lab[sel] - c * vs).astype(np.int32)
        g_tok[:len(sel)] = sel.astype(np.int32)
        g_msk[:len(sel)] = 1.0
        e_c = e_pad[c * vs:(c + 1) * vs]
        b_c = b_pad[c * vs:(c + 1) * vs]
        # exp bias per vocab-partition, with the -ln8 overflow guard; the
        # pad bias stays hugely negative so exp() of pad vocab is 0.
        bpp = b_c.reshape(n_vt, P).T.copy()
        bpp[bpp > BIAS_PAD / 2] -= LN8
        in_maps.append({
            "ht8": ht8,
            "et8": np.ascontiguousarray(e_c.T * BETA).astype(fp8),
            "bpp": np.ascontiguousarray(bpp),
            "h": h,
            "e": np.ascontiguousarray(e_c),
            "b": np.ascontiguousarray(b_c),
            "g_lbl": g_lbl.reshape(n_gtiles, P),
            "g_tok": g_tok.reshape(n_gtiles, P),
            "g_mask": g_msk.reshape(n_gtiles, P),
        })
    meta = dict(n_tok=n_tok, d_model=d_model, vs=vs, n_gtiles=n_gtiles,
                n_cores=n_cores)
    return in_maps, meta


_KERNEL_CACHE = {}
VARIANT = "d"


def _get_kernel(meta, variant=None):
    if variant is None:
        variant = VARIANT
    key = tuple(sorted(meta.items())) + (variant,)
    if key not in _KERNEL_CACHE:
        build = {"c": build_ce_kernel_c, "d": build_ce_kernel_d}[variant]
        _KERNEL_CACHE[key] = build(**meta)
    return _KERNEL_CACHE[key]


def kernel(outputs, word_embeddings, word_biases, labels):
    from concourse.bass_utils import run_bass_kernel_spmd

    in_maps, meta = host_prepare(outputs, word_embeddings, word_biases,
                                 labels, n_cores=N_CORES, vs=VS)
    nc = _get_kernel(meta)
    res = run_bass_kernel_spmd(nc, in_maps, list(range(meta["n_cores"])))
    loss = res.results[0]["loss"][0, 0]
    return np.float32(loss)



# revision 6
# speedup vs baseline: 1.0506x; 1.0413x over previous
"""Distributed cross-entropy loss kernel for Trainium2 (8 NeuronCores).

loss = -mean_t(log_softmax(h @ E^T + b)[t, labels[t]])
     = mean_t(LSE_t) - mean_t(h_t . E[labels[t]] + b[labels[t]])

Strategy (variant c): vocab tensor parallel across 8 cores; the host
pre-transposes and pre-quantizes both matmul operands to fp8e4 so the
device does nothing but fp8 DoubleRow matmuls, per-partition-biased
exp on the scalar engine, and fp8 DoubleRow ones-matmul partition
reductions. A tiny AllReduce combines per-core sumexp/target partials
and every core finishes the log + mean locally.

Orientation: vocab on PSUM partitions, tokens moving. Per (vocab-tile,
token-group) the PE accumulates K=1024 in 4 DoubleRow passes, ScalarE
applies exp(logit + b_v - ln8) writing fp8, and pairs of vocab tiles
are reduced over partitions with a DoubleRow ones-matmul into 8
persistent [1,512] PSUM accumulators (3 banks, offsets 0/32/64).

exp is pre-scaled by 1/8 (bias shift of -ln 8) so values fit e4m3's
max of 448; the final loss adds ln 8 back.
"""

from contextlib import ExitStack

import numpy as np
import ml_dtypes

import concourse.bass as bass
import concourse.tile as tile
from concourse import bacc, mybir

F32 = mybir.dt.float32
BF16 = mybir.dt.bfloat16
FP8 = mybir.dt.float8e4
I32 = mybir.dt.int32
AF = mybir.ActivationFunctionType
ALU = mybir.AluOpType
DR = mybir.MatmulPerfMode.DoubleRow

P = 128

# fp8 operand scaling: h' = ALPHA*h, E' = BETA*E with ALPHA*BETA == 1, so
# logits keep their true scale. Balancing puts both operands at ~0.18 std,
# inside e4m3's normal range (h ~ N(0,1), E rows ~ N(0, 1/D), D=1024).
BETA = 32.0 ** 0.5
ALPHA = 1.0 / BETA
LN8 = float(np.log(8.0))

# Problem constants (hardcoded per the harness contract).
B, T, D, V = 2, 2048, 1024, 50257
N_TOK = B * T
N_CORES = 8
VS = 6400                 # per-core padded vocab shard (8 * 6400 = 51200 >= V)
BIAS_PAD = -10000.0       # exp(x + BIAS_PAD) == 0 in fp32 for any real logit


def build_ce_kernel_c(n_tok, d_model, vs, n_gtiles, n_cores):
    """Variant c: host-pretransposed fp8 operands, PE-dense main loop."""
    n_dt = d_model // P        # K tiles of 128
    n_vt = vs // P             # vocab tiles of 128
    n_tb = n_tok // 512        # token blocks of 512 (s accumulator slots)
    n_tg = n_tok // 1024       # token groups of 1024 (PSUM mm granularity)
    assert n_dt % 2 == 0 and n_vt % 2 == 0 and n_tok % 1024 == 0

    nc = bacc.Bacc("TRN2", target_bir_lowering=False, debug=False,
                   num_devices=n_cores)

    # fp8 matmul operands, pre-transposed/scaled/padded on the host
    ht8_in = nc.dram_tensor("ht8", [d_model, n_tok], FP8, kind="ExternalInput")
    et8_in = nc.dram_tensor("et8", [d_model, vs], FP8, kind="ExternalInput")
    # per-partition exp bias: bias_pp[p, vt] = b[vt*128+p] - ln8 (or pad)
    bpp_in = nc.dram_tensor("bpp", [P, n_vt], F32, kind="ExternalInput")
    # fp32 copies for the target-logit gather path
    h_in = nc.dram_tensor("h", [n_tok, d_model], F32, kind="ExternalInput")
    e_in = nc.dram_tensor("e", [vs, d_model], F32, kind="ExternalInput")
    b_in = nc.dram_tensor("b", [vs], F32, kind="ExternalInput")
    gl_in = nc.dram_tensor("g_lbl", [n_gtiles, P], I32, kind="ExternalInput")
    gt_in = nc.dram_tensor("g_tok", [n_gtiles, P], I32, kind="ExternalInput")
    gm_in = nc.dram_tensor("g_mask", [n_gtiles, P], F32, kind="ExternalInput")
    loss_out = nc.dram_tensor("loss", [1, 1], F32, kind="ExternalOutput")

    # allreduce payload: n_tok sumexp/8 values + 128 tgt partials
    cc_len = n_tok + P
    cc_in = nc.dram_tensor("cc_in", [cc_len], F32)
    cc_out = nc.dram_tensor("cc_out", [cc_len], F32, addr_space="Shared")

    with tile.TileContext(nc, num_cores=n_cores) as tc:
        with ExitStack() as ctx:
            const = ctx.enter_context(tc.tile_pool(name="const", bufs=1))
            big_pool = ctx.enter_context(tc.tile_pool(name="big", bufs=1))
            exp_pool = ctx.enter_context(tc.tile_pool(name="expp", bufs=2))
            g_pool = ctx.enter_context(tc.tile_pool(name="g", bufs=6))
            fin_pool = ctx.enter_context(tc.tile_pool(name="fin", bufs=1))
            acc_pool = ctx.enter_context(tc.tile_pool(name="accp", bufs=1))
            mm_psum = ctx.enter_context(
                tc.tile_pool(name="mm_psum", bufs=2, space="PSUM"))

            # ---- constants ----
            ones1b = const.tile([P, 1], BF16)     # ones for final vocab sum
            nc.vector.memset(ones1b[:], 1.0)
            ones128 = const.tile([P, 1], F32)
            nc.vector.memset(ones128[:], 1.0)
            nones128 = const.tile([P, 1], F32)
            nc.vector.memset(nones128[:], -1.0)
            zbias = const.tile([P, 1], F32)
            nc.vector.memset(zbias[:], 0.0)
            bias_pp = const.tile([P, n_vt], F32)
            nc.sync.dma_start(bias_pp[:], bpp_in[:, :])

            # ---- fp8 operands straight from DRAM (no on-device prep) ----
            # layout [p, dc, x] with d = dc*128 + p; same factorization for
            # both operands so the contraction lines up.
            ht8 = big_pool.tile([P, n_dt, n_tok], FP8)
            ht8_src = ht8_in.rearrange("(dc p) t -> p dc t", p=P)
            tchunk = 1024
            for t0 in range(0, n_tok, tchunk):
                nc.sync.dma_start(ht8[:, :, t0:t0 + tchunk],
                                  ht8_src[:, :, t0:t0 + tchunk])
            et8 = big_pool.tile([P, n_dt, vs], FP8)
            et8_src = et8_in.rearrange("(dc p) v -> p dc v", p=P)
            vchunk = 640 if vs % 640 == 0 else P
            for v0 in range(0, vs, vchunk):
                nc.sync.dma_start(et8[:, :, v0:v0 + vchunk],
                                  et8_src[:, :, v0:v0 + vchunk])

            # ---- target logits (gather path; overlaps the main loop) ----
            lbl_sb = fin_pool.tile([P, n_gtiles], I32)
            tok_sb = fin_pool.tile([P, n_gtiles], I32)
            msk_sb = fin_pool.tile([P, n_gtiles], F32)
            nc.sync.dma_start(lbl_sb[:], gl_in.rearrange("g p -> p g"))
            nc.sync.dma_start(tok_sb[:], gt_in.rearrange("g p -> p g"))
            nc.sync.dma_start(msk_sb[:], gm_in.rearrange("g p -> p g"))
            dots = fin_pool.tile([P, n_gtiles], F32)
            bg = fin_pool.tile([P, n_gtiles], F32)
            for g in range(n_gtiles):
                eg = g_pool.tile([P, d_model], F32, tag="grow")
                nc.gpsimd.indirect_dma_start(
                    out=eg[:], out_offset=None, in_=e_in[:, :],
                    in_offset=bass.IndirectOffsetOnAxis(
                        ap=lbl_sb[:, g:g + 1], axis=0))
                hg = g_pool.tile([P, d_model], F32, tag="grow")
                nc.gpsimd.indirect_dma_start(
                    out=hg[:], out_offset=None, in_=h_in[:, :],
                    in_offset=bass.IndirectOffsetOnAxis(
                        ap=tok_sb[:, g:g + 1], axis=0))
                nc.gpsimd.indirect_dma_start(
                    out=bg[:, g:g + 1], out_offset=None,
                    in_=b_in.rearrange("(v o) -> v o", o=1),
                    in_offset=bass.IndirectOffsetOnAxis(
                        ap=lbl_sb[:, g:g + 1], axis=0))
                gsc = g_pool.tile([P, d_model], F32, tag="grow")
                nc.vector.tensor_mul(gsc[:], eg[:], hg[:])
                nc.vector.tensor_reduce(
                    dots[:, g:g + 1], gsc[:],
                    axis=mybir.AxisListType.X, op=ALU.add)
            dsum = fin_pool.tile([P, n_gtiles], F32)
            nc.vector.tensor_add(dsum[:], dots[:], bg[:])
            dmask = fin_pool.tile([P, n_gtiles], F32)
            nc.vector.tensor_mul(dmask[:], dsum[:], msk_sb[:])
            tgt_red = fin_pool.tile([P, 1], F32)
            nc.vector.tensor_reduce(
                tgt_red[:], dmask[:], axis=mybir.AxisListType.X, op=ALU.add)

            # ---- main loop: token super-groups outer, vocab inner ----
            # Both fp8 operands are SBUF-resident. Per super-group (TS
            # tokens = one 4-bank PSUM tile) the PE streams 100%
            # homogeneous fp8 DoubleRow matmuls; ScalarE does one wide
            # exp per vocab tile; the DVE accumulates exp tiles into an
            # SBUF f32 accumulator (vocab partition-sums happen once, at
            # the tail, via cheap bf16 ones-matmuls).
            TS = 2048 if n_tok % 2048 == 0 else n_tok
            n_ts = n_tok // TS
            nk = TS // 512
            accs = []
            for ts in range(n_ts):
                acc = acc_pool.tile([P, TS], F32, name=f"acc{ts}")
                accs.append(acc)
                for vt in range(n_vt):
                    ps = mm_psum.tile([P, TS], F32, tag="mm",
                                      name=f"ps{ts}_{vt}")
                    for j in range(n_dt // 2):
                        for k in range(nk):
                            nc.tensor.matmul(
                                ps[:, k * 512:(k + 1) * 512],
                                lhsT=et8[:, 2 * j:2 * j + 2,
                                         vt * P:(vt + 1) * P],
                                rhs=ht8[:, 2 * j:2 * j + 2,
                                        ts * TS + k * 512:
                                        ts * TS + (k + 1) * 512],
                                start=(j == 0), stop=(j == n_dt // 2 - 1),
                                perf_mode=DR)
                    ex = exp_pool.tile([P, TS], BF16, tag="exp",
                                       name=f"exp{ts}_{vt}")
                    nc.scalar.activation(
                        ex[:], ps[:], AF.Exp, bias=bias_pp[:, vt:vt + 1])
                    if vt == 0:
                        nc.vector.tensor_copy(acc[:], ex[:])
                    else:
                        nc.vector.tensor_add(acc[:], acc[:], ex[:])

            # ---- tail: vocab partition-sums + ship partials ----
            s_sbs = [fin_pool.tile([P, 512], F32, name=f"s_sb{i}")
                     for i in range(-(-n_tb // 4))]

            def s_row(tb):
                return s_sbs[tb // 4][32 * (tb % 4):32 * (tb % 4) + 1, :]

            for ts in range(n_ts):
                accb = exp_pool.tile([P, TS], BF16, tag="exp",
                                     name=f"accb{ts}")
                nc.vector.tensor_copy(accb[:], accs[ts][:])
                red = mm_psum.tile([P, TS], F32, tag="mm", name=f"red{ts}")
                for k in range(nk):
                    tb = ts * nk + k
                    nc.tensor.matmul(
                        red[0:1, k * 512:(k + 1) * 512], lhsT=ones1b[:],
                        rhs=accb[:, k * 512:(k + 1) * 512],
                        start=True, stop=True, skip_group_check=True)
                for k in range(nk):
                    tb = ts * nk + k
                    nc.vector.tensor_copy(
                        s_row(tb), red[0:1, k * 512:(k + 1) * 512])
                    nc.sync.dma_start(
                        cc_in[tb * 512:(tb + 1) * 512].rearrange(
                            "(a b) -> a b", a=1),
                        s_row(tb))

            # ---- allreduce S partials + tgt partials ----
            nc.sync.dma_start(
                cc_in[n_tok:cc_len].rearrange("(a b) -> a b", a=P),
                tgt_red[:])
            nc.gpsimd.collective_compute(
                "AllReduce", ALU.add,
                replica_groups=[list(range(n_cores))],
                ins=[cc_in.rearrange("(a b) -> a b", a=8)],
                outs=[cc_out.rearrange("(a b) -> a b", a=8)])
            s_glob = fin_pool.tile([n_tb, 512], F32)
            nc.sync.dma_start(
                s_glob[:], cc_out[0:n_tok].rearrange("(a b) -> a b", a=n_tb))
            tgt_glob = fin_pool.tile([P, 1], F32)
            nc.sync.dma_start(
                tgt_glob[:],
                cc_out[n_tok:cc_len].rearrange("(a b) -> a b", a=P))

            # ---- loss = mean(ln S') + ln 8 - mean(tgt) ----
            lse = fin_pool.tile([n_tb, 512], F32)
            lse_sum = fin_pool.tile([n_tb, 1], F32)
            nc.scalar.activation(
                lse[:], s_glob[:], AF.Ln, bias=zbias[0:n_tb, :],
                accum_out=lse_sum[:])
            lp = mm_psum.tile([1, 1], F32, tag="mm")
            nc.tensor.matmul(lp[:], lhsT=ones128[0:n_tb, :],
                             rhs=lse_sum[:], start=True, stop=False,
                             skip_group_check=True)
            nc.tensor.matmul(lp[:], lhsT=nones128[:], rhs=tgt_glob[:],
                             start=False, stop=True, skip_group_check=True)
            loss_sb = fin_pool.tile([1, 1], F32)
            nc.scalar.activation(loss_sb[:], lp[:], AF.Copy,
                                 scale=1.0 / float(n_tok), bias=LN8)
            nc.sync.dma_start(loss_out[:, :], loss_sb[:])

    nc.finalize()
    return nc


def build_ce_kernel_d(n_tok, d_model, vs, n_gtiles, n_cores):
    """Variant d: variant c plus
    - chunked et8/ht8 input tiles so the PE starts ~8us in (not ~30us)
    - bf16 accumulator (DVE 2x_1p mode, and no tail copy to bf16)
    - per-ts split AllReduce: ts0's collective (+ target-logit payload)
      overlaps ts1 compute; only ts1's small collective is exposed
    """
    n_dt = d_model // P        # K tiles of 128
    n_vt = vs // P             # vocab tiles of 128
    n_tb = n_tok // 512        # token blocks of 512
    assert n_dt % 2 == 0 and n_tok % 1024 == 0

    TS = 2048 if n_tok % 2048 == 0 else n_tok
    n_ts = n_tok // TS
    nk = TS // 512

    # input chunk sizes
    TOK_CH = 1024
    n_ht = n_tok // TOK_CH
    CH_V = 640 if vs % 640 == 0 else vs
    n_ec = vs // CH_V
    ch_vt = CH_V // P          # vocab tiles per et8 chunk

    nc = bacc.Bacc("TRN2", target_bir_lowering=False, debug=False,
                   num_devices=n_cores)

    ht8_in = nc.dram_tensor("ht8", [d_model, n_tok], FP8, kind="ExternalInput")
    et8_in = nc.dram_tensor("et8", [d_model, vs], FP8, kind="ExternalInput")
    bpp_in = nc.dram_tensor("bpp", [P, n_vt], F32, kind="ExternalInput")
    h_in = nc.dram_tensor("h", [n_tok, d_model], F32, kind="ExternalInput")
    e_in = nc.dram_tensor("e", [vs, d_model], F32, kind="ExternalInput")
    b_in = nc.dram_tensor("b", [vs], F32, kind="ExternalInput")
    gl_in = nc.dram_tensor("g_lbl", [n_gtiles, P], I32, kind="ExternalInput")
    gt_in = nc.dram_tensor("g_tok", [n_gtiles, P], I32, kind="ExternalInput")
    gm_in = nc.dram_tensor("g_mask", [n_gtiles, P], F32, kind="ExternalInput")
    loss_out = nc.dram_tensor("loss", [1, 1], F32, kind="ExternalOutput")

    # collective payloads: cc0 = ts0 sumexp + tgt partials; cc1 = rest
    cc0_len = TS + P
    cc0_in = nc.dram_tensor("cc0_in", [cc0_len], F32)
    cc0_out = nc.dram_tensor("cc0_out", [cc0_len], F32, addr_space="Shared")
    if n_ts > 1:
        cc1_len = (n_ts - 1) * TS
        cc1_in = nc.dram_tensor("cc1_in", [cc1_len], F32)
        cc1_out = nc.dram_tensor("cc1_out", [cc1_len], F32,
                                 addr_space="Shared")

    with tile.TileContext(nc, num_cores=n_cores) as tc:
        with ExitStack() as ctx:
            const = ctx.enter_context(tc.tile_pool(name="const", bufs=1))
            big_pool = ctx.enter_context(tc.tile_pool(name="big", bufs=1))
            exp_pool = ctx.enter_context(tc.tile_pool(name="expp", bufs=2))
            g_pool = ctx.enter_context(tc.tile_pool(name="g", bufs=6))
            fin_pool = ctx.enter_context(tc.tile_pool(name="fin", bufs=1))
            acc_pool = ctx.enter_context(tc.tile_pool(name="accp", bufs=1))
            mm_psum = ctx.enter_context(
                tc.tile_pool(name="mm_psum", bufs=2, space="PSUM"))

            # ---- constants ----
            ones1b = const.tile([P, 1], BF16)
            nc.vector.memset(ones1b[:], 1.0)
            ones128 = const.tile([P, 1], F32)
            nc.vector.memset(ones128[:], 1.0)
            nones128 = const.tile([P, 1], F32)
            nc.vector.memset(nones128[:], -1.0)
            zbias = const.tile([P, 1], F32)
            nc.vector.memset(zbias[:], 0.0)
            bias_pp = const.tile([P, n_vt], F32)
            nc.sync.dma_start(bias_pp[:], bpp_in[:, :])

            # ---- chunked fp8 operand loads, ordered for earliest start ----
            ht8_src = ht8_in.rearrange("(dc p) t -> p dc t", p=P)
            et8_src = et8_in.rearrange("(dc p) v -> p dc v", p=P)
            et_c = [big_pool.tile([P, n_dt, CH_V], FP8, name=f"et{c}")
                    for c in range(n_ec)]
            ht_t = [big_pool.tile([P, n_dt, TOK_CH], FP8, name=f"ht{i}")
                    for i in range(n_ht)]
            nc.sync.dma_start(et_c[0][:],
                              et8_src[:, :, 0:CH_V])
            for i in range(min(2, n_ht)):
                nc.sync.dma_start(ht_t[i][:],
                                  ht8_src[:, :, i * TOK_CH:(i + 1) * TOK_CH])
            for c in range(1, n_ec):
                nc.sync.dma_start(et_c[c][:],
                                  et8_src[:, :, c * CH_V:(c + 1) * CH_V])
            for i in range(2, n_ht):
                nc.sync.dma_start(ht_t[i][:],
                                  ht8_src[:, :, i * TOK_CH:(i + 1) * TOK_CH])
            # gather-path metadata last: delays the indirect row gathers
            # past the hot et8/ht8 stream (they only must beat ts0's end)
            lbl_sb = fin_pool.tile([P, n_gtiles], I32)
            tok_sb = fin_pool.tile([P, n_gtiles], I32)
            msk_sb = fin_pool.tile([P, n_gtiles], F32)
            nc.sync.dma_start(lbl_sb[:], gl_in.rearrange("g p -> p g"))
            nc.sync.dma_start(tok_sb[:], gt_in.rearrange("g p -> p g"))
            nc.sync.dma_start(msk_sb[:], gm_in.rearrange("g p -> p g"))

            # ---- target logits (gather path; overlaps the main loop) ----
            dots = fin_pool.tile([P, n_gtiles], F32)
            bg = fin_pool.tile([P, n_gtiles], F32)
            for g in range(n_gtiles):
                eg = g_pool.tile([P, d_model], F32, tag="grow")
                nc.gpsimd.indirect_dma_start(
                    out=eg[:], out_offset=None, in_=e_in[:, :],
                    in_offset=bass.IndirectOffsetOnAxis(
                        ap=lbl_sb[:, g:g + 1], axis=0))
                hg = g_pool.tile([P, d_model], F32, tag="grow")
                nc.gpsimd.indirect_dma_start(
                    out=hg[:], out_offset=None, in_=h_in[:, :],
                    in_offset=bass.IndirectOffsetOnAxis(
                        ap=tok_sb[:, g:g + 1], axis=0))
                nc.gpsimd.indirect_dma_start(
                    out=bg[:, g:g + 1], out_offset=None,
                    in_=b_in.rearrange("(v o) -> v o", o=1),
                    in_offset=bass.IndirectOffsetOnAxis(
                        ap=lbl_sb[:, g:g + 1], axis=0))
                gsc = g_pool.tile([P, d_model], F32, tag="grow")
                nc.vector.tensor_mul(gsc[:], eg[:], hg[:])
                nc.vector.tensor_reduce(
                    dots[:, g:g + 1], gsc[:],
                    axis=mybir.AxisListType.X, op=ALU.add)
            dsum = fin_pool.tile([P, n_gtiles], F32)
            nc.vector.tensor_add(dsum[:], dots[:], bg[:])
            dmask = fin_pool.tile([P, n_gtiles], F32)
            nc.vector.tensor_mul(dmask[:], dsum[:], msk_sb[:])
            tgt_red = fin_pool.tile([P, 1], F32)
            nc.vector.tensor_reduce(
                tgt_red[:], dmask[:], axis=mybir.AxisListType.X, op=ALU.add)

            # ---- final-reduction tiles ----
            s_glob = fin_pool.tile([n_tb, 512], F32)
            tgt_glob = fin_pool.tile([P, 1], F32)
            s_sbs = [fin_pool.tile([1, TS], F32, name=f"s_sb{t}")
                     for t in range(n_ts)]

            # ---- main loop ----
            for ts in range(n_ts):
                acc = acc_pool.tile([P, TS], BF16, name=f"acc{ts}")
                for vt in range(n_vt):
                    ec, vl = vt // ch_vt, vt % ch_vt
                    ps = mm_psum.tile([P, TS], F32, tag="mm",
                                      name=f"ps{ts}_{vt}")
                    for j in range(n_dt // 2):
                        for k in range(nk):
                            g0 = ts * TS + k * 512
                            ti, toff = g0 // TOK_CH, g0 % TOK_CH
                            nc.tensor.matmul(
                                ps[:, k * 512:(k + 1) * 512],
                                lhsT=et_c[ec][:, 2 * j:2 * j + 2,
                                              vl * P:(vl + 1) * P],
                                rhs=ht_t[ti][:, 2 * j:2 * j + 2,
                                             toff:toff + 512],
                                start=(j == 0), stop=(j == n_dt // 2 - 1),
                                perf_mode=DR)
                    if vt == 0:
                        nc.scalar.activation(
                            acc[:], ps[:], AF.Exp, bias=bias_pp[:, 0:1])
                    else:
                        ex = exp_pool.tile([P, TS], BF16, tag="exp",
                                           name=f"exp{ts}_{vt}")
                        nc.scalar.activation(
                            ex[:], ps[:], AF.Exp, bias=bias_pp[:, vt:vt + 1])
                        nc.vector.tensor_add(acc[:], acc[:], ex[:])

                # ---- ts tail: vocab partition-sum, ship, collective ----
                # per-512 copies pipeline behind the ones-matmuls so the
                # borrowed PSUM slot frees sooner (shorter hot-loop stall)
                red = mm_psum.tile([P, TS], F32, tag="mm", name=f"red{ts}")
                for k in range(nk):
                    nc.tensor.matmul(
                        red[0:1, k * 512:(k + 1) * 512], lhsT=ones1b[:],
                        rhs=acc[:, k * 512:(k + 1) * 512],
                        start=True, stop=True, skip_group_check=True)
                for k in range(nk):
                    nc.vector.tensor_copy(
                        s_sbs[ts][0:1, k * 512:(k + 1) * 512],
                        red[0:1, k * 512:(k + 1) * 512])
                if ts == 0:
                    nc.sync.dma_start(
                        cc0_in[0:TS].rearrange("(a b) -> a b", a=1),
                        s_sbs[0][:])
                    nc.sync.dma_start(
                        cc0_in[TS:cc0_len].rearrange("(a b) -> a b", a=P),
                        tgt_red[:])
                    nc.gpsimd.collective_compute(
                        "AllReduce", ALU.add,
                        replica_groups=[list(range(n_cores))],
                        ins=[cc0_in.rearrange("(a b) -> a b", a=8)],
                        outs=[cc0_out.rearrange("(a b) -> a b", a=8)])
                    nc.sync.dma_start(
                        s_glob[0:nk, :],
                        cc0_out[0:TS].rearrange("(a b) -> a b", a=nk))
                    nc.sync.dma_start(
                        tgt_glob[:],
                        cc0_out[TS:cc0_len].rearrange("(a b) -> a b", a=P))
                else:
                    o = (ts - 1) * TS
                    nc.sync.dma_start(
                        cc1_in[o:o + TS].rearrange("(a b) -> a b", a=1),
                        s_sbs[ts][:])

            if n_ts > 1:
                nc.gpsimd.collective_compute(
                    "AllReduce", ALU.add,
                    replica_groups=[list(range(n_cores))],
                    ins=[cc1_in.rearrange("(a b) -> a b", a=8)],
                    outs=[cc1_out.rearrange("(a b) -> a b", a=8)])
                nc.sync.dma_start(
                    s_glob[nk:n_tb, :],
                    cc1_out[:].rearrange("(a b) -> a b", a=n_tb - nk))

            # hide the Ln activation-table load under the last collective:
            # this dummy has no data deps on the collective result, so the
            # scalar engine loads the Ln table while CC waits
            ln_warm = fin_pool.tile([1, 1], F32)
            nc.scalar.activation(ln_warm[:], ones128[0:1, :], AF.Ln,
                                 bias=zbias[0:1, :])

            # ---- loss = mean(ln S') + ln 8 - mean(tgt) ----
            lse = fin_pool.tile([n_tb, 512], F32)
            lse_sum = fin_pool.tile([n_tb, 1], F32)
            nc.scalar.activation(
                lse[:], s_glob[:], AF.Ln, bias=zbias[0:n_tb, :],
                accum_out=lse_sum[:])
            lp = mm_psum.tile([1, 1], F32, tag="mm")
            nc.tensor.matmul(lp[:], lhsT=ones128[0:n_tb, :],
                             rhs=lse_sum[:], start=True, stop=False,
                             skip_group_check=True)
            nc.tensor.matmul(lp[:], lhsT=nones128[:], rhs=tgt_glob[:],
                             start=False, stop=True, skip_group_check=True)
            loss_sb = fin_pool.tile([1, 1], F32)
            nc.scalar.activation(loss_sb[:], lp[:], AF.Copy,
                                 scale=1.0 / float(n_tok), bias=LN8)
            nc.sync.dma_start(loss_out[:, :], loss_sb[:])

    nc.finalize()
    return nc


def host_prepare(outputs, word_embeddings, word_biases, labels,
                 n_cores=N_CORES, vs=None):
    """Shard/pad/transpose/quantize the full inputs into per-core maps."""
    d_model = outputs.shape[-1]
    v_real = word_embeddings.shape[0]
    n_tok = outputs.shape[0] * outputs.shape[1]
    if vs is None:
        vs = -(-v_real // (n_cores * 2 * P)) * 2 * P  # per-core, mult of 256
    v_pad = n_cores * vs
    n_vt = vs // P

    h = np.ascontiguousarray(
        np.asarray(outputs, dtype=np.float32).reshape(n_tok, d_model))
    e_pad = np.zeros((v_pad, d_model), dtype=np.float32)
    e_pad[:v_real] = np.asarray(word_embeddings, dtype=np.float32)
    b_pad = np.full(v_pad, BIAS_PAD, dtype=np.float32)
    b_pad[:v_real] = np.asarray(word_biases, dtype=np.float32)
    lab = np.asarray(labels).reshape(-1).astype(np.int64)

    fp8 = ml_dtypes.float8_e4m3
    ht8 = np.ascontiguousarray((h.T * ALPHA)).astype(fp8)

    # Per-core gather lists: labels that fall inside each core's shard.
    sels = [np.nonzero((lab >= c * vs) & (lab < (c + 1) * vs))[0]
            for c in range(n_cores)]
    cap = max(max((len(s) for s in sels), default=1), 1)
    n_gtiles = -(-cap // P)
    gcap = n_gtiles * P

    in_maps = []
    for c in range(n_cores):
        sel = sels[c]
        g_lbl = np.zeros(gcap, dtype=np.int32)
        g_tok = np.zeros(gcap, dtype=np.int32)
        g_msk = np.zeros(gcap, dtype=np.float32)
        g_lbl[:len(sel)] = (lab[sel] - c * vs).astype(np.int32)
        g_tok[:len(sel)] = sel.astype(np.int32)
        g_msk[:len(sel)] = 1.0
        e_c = e_pad[c * vs:(c + 1) * vs]
        b_c = b_pad[c * vs:(c + 1) * vs]
        # exp bias per vocab-partition, with the -ln8 overflow guard; the
        # pad bias stays hugely negative so exp() of pad vocab is 0.
        bpp = b_c.reshape(n_vt, P).T.copy()
        bpp[bpp > BIAS_PAD / 2] -= LN8
        in_maps.append({
            "ht8": ht8,
            "et8": np.ascontiguousarray(e_c.T * BETA).astype(fp8),
            "bpp": np.ascontiguousarray(bpp),
            "h": h,
            "e": np.ascontiguousarray(e_c),
            "b": np.ascontiguousarray(b_c),
            "g_lbl": g_lbl.reshape(n_gtiles, P),
            "g_tok": g_tok.reshape(n_gtiles, P),
            "g_mask": g_msk.reshape(n_gtiles, P),
        })
    meta = dict(n_tok=n_tok, d_model=d_model, vs=vs, n_gtiles=n_gtiles,
                n_cores=n_cores)
    return in_maps, meta


_KERNEL_CACHE = {}
VARIANT = "d"


def _get_kernel(meta, variant=None):
    if variant is None:
        variant = VARIANT
    key = tuple(sorted(meta.items())) + (variant,)
    if key not in _KERNEL_CACHE:
        build = {"c": build_ce_kernel_c, "d": build_ce_kernel_d}[variant]
        _KERNEL_CACHE[key] = build(**meta)
    return _KERNEL_CACHE[key]


def kernel(outputs, word_embeddings, word_biases, labels):
    from concourse.bass_utils import run_bass_kernel_spmd

    in_maps, meta = host_prepare(outputs, word_embeddings, word_biases,
                                 labels, n_cores=N_CORES, vs=VS)
    nc = _get_kernel(meta)
    res = run_bass_kernel_spmd(nc, in_maps, list(range(meta["n_cores"])))
    loss = res.results[0]["loss"][0, 0]
    return np.float32(loss)

